# revision 23
# baseline (speedup 1.0000x reference)
"""Trainium2 Bass kernel for GQA attention (B=2, S=2048, D=2048, H=16, HK=4).

Sharding: 8 devices = batch(2) x kv-groups(4). Each device owns one batch
element and one GQA group (4 q-heads + 1 kv-head): wq/wk/wv column-parallel,
wo row-parallel (host sums the 4 partials per batch element).

Device kernel (all matmuls fp32r):
  - QKV projection in [dk, s] orientation (weights stationary, xT moving)
  - RoPE on DVE with an even/odd dk permutation folded into the weights
    host-side (partitions 0:64 = real, 64:128 = imag)
  - scores computed transposed [sk, sq] so softmax needs no transposes;
    exp on ACT reads PSUM directly; no max-subtraction (scores are O(1)
    for this problem's scale)
  - denominators via M=1 ones-matmuls packed 2-per-PSUM-bank
  - causal: upper-triangle score tiles skipped entirely; diagonal tiles
    get an additive -1e9 mask before exp
  - wo row-parallel matmul on device; host adds wo_b and reduces groups
"""

import math

import numpy as np

import concourse.bass as bass
import concourse.bacc as bacc
import concourse.tile as tile
from concourse import mybir
from concourse.bass_utils import run_bass_kernel_spmd

B, S, D = 2, 2048, 2048
H, HK, DK = 16, 4, 128
REP = H // HK  # 4 q-heads per kv head
NDEV = 8
P = 128
CH = 512            # s-chunk (matmul moving size)
ND = D // P         # 16 d-tiles
F32 = mybir.dt.float32
F32R = mybir.dt.float32r
NEG = -1.0e9


def _build(s_len=S, reps=1):
    """Build the per-device Bass program (SPMD: same program on all cores).

    reps>1 repeats the whole computation (timing only)."""
    nch = s_len // CH          # s-chunks
    nt = s_len // P            # sk tiles
    hf_cnt = max(1, s_len // 1024)   # s-halves for x residency
    sh = s_len // hf_cnt       # s elems per half
    cph = sh // CH             # chunks per half
    tph = sh // P              # sk tiles per half
    scale = 1.0 / math.sqrt(DK)

    nc = bacc.Bacc("TRN2", target_bir_lowering=False, debug=False,
                   enable_asserts=False, num_devices=1)
    xT = nc.dram_tensor("xT", [D, s_len], F32R, kind="ExternalInput").ap()
    W = nc.dram_tensor("W", [P, 6 * D], F32R, kind="ExternalInput").ap()
    woT = nc.dram_tensor("woT", [REP * DK, D], F32R, kind="ExternalInput").ap()
    CSt = nc.dram_tensor("CS", [P, s_len], F32, kind="ExternalInput").ap()
    SCt = nc.dram_tensor("SC", [P, s_len], F32, kind="ExternalInput").ap()
    MBt = nc.dram_tensor("MB", [P, 896], F32, kind="ExternalInput").ap()
    IDt = nc.dram_tensor("ID", [P, P], F32, kind="ExternalInput").ap()
    BIAS = nc.dram_tensor("BIAS", [P, 6], F32, kind="ExternalInput").ap()
    BIAS2 = nc.dram_tensor("BIAS2", [P, 6], F32, kind="ExternalInput").ap()
    ONEH = nc.dram_tensor("ONEH", [P, 4 * REP], F32R, kind="ExternalInput").ap()
    ONEH4 = nc.dram_tensor("ONEH4", [REP, REP * P], F32R, kind="ExternalInput").ap()
    out = nc.dram_tensor("out", [s_len, D], F32, kind="ExternalOutput").ap()

    with tile.TileContext(nc) as tc:
      for _rep in range(reps):
        with tc.tile_pool(name="consts", bufs=1) as consts, \
             tc.tile_pool(name="qkv", bufs=1) as qkpool:
            cs_sb = consts.tile([P, s_len], F32)
            sc_sb = consts.tile([P, s_len], F32)
            mb_sb = consts.tile([P, 896], F32)
            id_sb = consts.tile([P, P], F32)
            bias_sb = consts.tile([P, 6], F32)
            bias2_sb = consts.tile([P, 6], F32)
            oneh_sb = consts.tile([P, 4 * REP], F32R)
            oneh4_sb = consts.tile([REP, REP * P], F32R)

            qk_sb = qkpool.tile([P, 5 * s_len], F32R)  # roped q heads 0..3, k at block 4
            v_sb = qkpool.tile([P, s_len], F32R)       # [s-in-tile, dk] tiles along free

            # ---------------- Phase P: QKV projection + RoPE + v transpose
            # SP DMA order: W[m=0], x[q=0], W[1..5], consts, x[q=1..3].
            # W fully resident (6 MB); x double-buffered in quarter tiles.
            with tc.tile_pool(name="xh", bufs=2) as xpool, \
                 tc.tile_pool(name="wst", bufs=1) as wpool, \
                 tc.tile_pool(name="rope", bufs=3) as rpool, \
                 tc.tile_pool(name="vT", bufs=1) as vtpool, \
                 tc.tile_pool(name="pp", bufs=4, space="PSUM") as pps, \
                 tc.tile_pool(name="pt", bufs=2, space="PSUM") as pts:
                vT_sb = vtpool.tile([P, s_len], F32)
                w_sb = wpool.tile([P, 6 * D], F32R)

                def load_xq(q, dts=None):
                    xq = xpool.tile([P, ND * CH], F32R, tag="x", name=f"xq{q}")
                    for dt in (range(ND) if dts is None else dts):
                        nc.sync.dma_start(
                            out=xq[:, dt * CH:(dt + 1) * CH],
                            in_=xT[dt * P:(dt + 1) * P, q * CH:(q + 1) * CH])
                    return xq

                def load_x_slices(xq, q, dts):
                    for dt in dts:
                        nc.sync.dma_start(
                            out=xq[:, dt * CH:(dt + 1) * CH],
                            in_=xT[dt * P:(dt + 1) * P, q * CH:(q + 1) * CH])

                def load_tabs(c):
                    nc.sync.dma_start(out=cs_sb[:, c * CH:(c + 1) * CH],
                                      in_=CSt[:, c * CH:(c + 1) * CH])
                    nc.sync.dma_start(out=sc_sb[:, c * CH:(c + 1) * CH],
                                      in_=SCt[:, c * CH:(c + 1) * CH])

                # interleave W[m=0] sub-loads with the first x slices so the
                # first k-step can issue ~2us in (each W chunk covers 4 dt)
                nc.sync.dma_start(out=w_sb[:, 0:CH], in_=W[:, 0:CH])
                xq = load_xq(0, dts=[0])
                for wc in range(1, 4):
                    load_x_slices(xq, 0, range((wc - 1) * 4 + 1, wc * 4 + 1))
                    nc.sync.dma_start(out=w_sb[:, wc * CH:(wc + 1) * CH],
                                      in_=W[:, wc * CH:(wc + 1) * CH])
                load_x_slices(xq, 0, range(13, ND))
                nc.sync.dma_start(out=bias_sb, in_=BIAS)
                nc.sync.dma_start(out=bias2_sb, in_=BIAS2)
                load_tabs(0)
                for m in range(1, 6):
                    nc.sync.dma_start(out=w_sb[:, m * D:(m + 1) * D],
                                      in_=W[:, m * D:(m + 1) * D])
                nc.sync.dma_start(out=id_sb, in_=IDt)

                for c in range(nch):
                    if c > 0:
                        xq = load_xq(c)
                        load_tabs(c)
                    if c == (1 if nch > 1 else 0):
                        nc.sync.dma_start(out=mb_sb, in_=MBt)
                        nc.sync.dma_start(out=oneh_sb, in_=ONEH)
                        nc.sync.dma_start(out=oneh4_sb, in_=ONEH4)
                    for m in range(6):
                        ps = pps.tile([P, CH], F32, tag="pp")
                        for dt in range(ND):
                            nc.tensor.matmul(
                                ps, w_sb[:, m * D + dt * P: m * D + (dt + 1) * P],
                                xq[:, dt * CH:(dt + 1) * CH],
                                start=(dt == 0), stop=(dt == ND - 1))
                        if m < 5:
                            # RoPE: partitions 0:64 real (qr), 64:128 imag (qi).
                            # U[0:64]=(qr+b0)cos  U[64:]=(qr+b0)sin
                            # V[0:64]=(qi+b1)sin  V[64:]=(qi+b1)cos
                            # All SBUF operands of each op share a base
                            # partition (walrus NCC_IBIR297); the PSUM
                            # input's base is independent.
                            cs_c = cs_sb[:, c * CH:(c + 1) * CH]
                            sc_c = sc_sb[:, c * CH:(c + 1) * CH]
                            add, mult = mybir.AluOpType.add, mybir.AluOpType.mult
                            u = rpool.tile([P, CH], F32, tag="p1")
                            v = rpool.tile([P, CH], F32, tag="p2")
                            nc.vector.scalar_tensor_tensor(
                                u[0:64], ps[0:64], bias_sb[0:64, m:m + 1],
                                cs_c[0:64], op0=add, op1=mult)
                            nc.vector.scalar_tensor_tensor(
                                u[64:128], ps[0:64], bias2_sb[64:128, m:m + 1],
                                cs_c[64:128], op0=add, op1=mult)
                            nc.vector.scalar_tensor_tensor(
                                v[0:64], ps[64:128], bias2_sb[0:64, m:m + 1],
                                sc_c[0:64], op0=add, op1=mult)
                            nc.vector.scalar_tensor_tensor(
                                v[64:128], ps[64:128], bias_sb[64:128, m:m + 1],
                                sc_c[64:128], op0=add, op1=mult)
                            dst = qk_sb[:, m * s_len + c * CH: m * s_len + (c + 1) * CH]
                            nc.vector.tensor_sub(dst[0:64], u[0:64], v[0:64])
                            nc.vector.tensor_add(dst[64:128], u[64:128], v[64:128])
                        else:
                            nc.scalar.add(out=vT_sb[:, c * CH:(c + 1) * CH],
                                          in_=ps, add=bias_sb[:, m:m + 1])
                    for tt in range(c * (CH // P), (c + 1) * (CH // P)):
                        ptr = pts.tile([P, P], F32, tag="pt")
                        nc.tensor.transpose(ptr, vT_sb[:, tt * P:(tt + 1) * P], id_sb)
                        nc.any.tensor_copy(v_sb[:, tt * P:(tt + 1) * P], ptr)

            # ---------------- Phase A: attention
            with tc.tile_pool(name="oh", bufs=1) as ohpool, \
                 tc.tile_pool(name="wo", bufs=1) as wopool:
                ohT_sb = ohpool.tile([P, REP * s_len], F32R)
                woT_sb = wopool.tile([P, REP * D], F32R)
                for j in range(REP):
                    nc.sync.dma_start(out=woT_sb[:, j * D:(j + 1) * D],
                                      in_=woT[j * P:(j + 1) * P, :])

                with tc.tile_pool(name="ew", bufs=8) as epool, \
                     tc.tile_pool(name="mt", bufs=4) as tpool, \
                     tc.tile_pool(name="nrm", bufs=4) as npool, \
                     tc.tile_pool(name="ps_s", bufs=2, space="PSUM") as pss, \
                     tc.tile_pool(name="ps_o", bufs=4, space="PSUM") as pso, \
                     tc.tile_pool(name="ps_b", bufs=1, space="PSUM") as psb, \
                     tc.tile_pool(name="ps_d", bufs=1, space="PSUM") as psd:
                    for c in range(nch):
                        nt_c = (c + 1) * (CH // P)  # causal sk tiles for this chunk
                        od = [pso.tile([P, CH], F32, tag="od", name=f"od{c}_{h}")
                              for h in range(REP)]
                        dd = psd.tile([REP, CH], F32, tag="dd")
                        for t in range(nt_c):
                            # causal: slice the moving (sq) range down to the
                            # valid region, >=256 wide (fp32r rate needs
                            # ap_size >= 256). delta = c*CH - t*P.
                            delta = c * CH - t * P
                            if delta >= 256:
                                f0, fn = 0, CH
                            elif delta >= 0:
                                f0, fn = 0, CH
                            elif delta == -128:
                                f0, fn = 128, 384
                            else:  # -256, -384
                                f0, fn = 256, 256
                            diag = (t >= nt_c - 4)
                            es = []
                            for h in range(REP):
                                ss = pss.tile([P, CH], F32, tag="sc")
                                nc.tensor.matmul(
                                    ss[:, 0:fn],
                                    qk_sb[:, 4 * s_len + t * P: 4 * s_len + (t + 1) * P],
                                    qk_sb[:, h * s_len + c * CH + f0: h * s_len + c * CH + f0 + fn],
                                    start=True, stop=True)
                                e = epool.tile([P, CH], F32R, tag="e")
                                if diag:
                                    off = delta + 384 + f0
                                    tmp = tpool.tile([P, CH], F32, tag="mt")
                                    nc.vector.scalar_tensor_tensor(
                                        tmp[:, 0:fn], ss[:, 0:fn], scale,
                                        mb_sb[:, off:off + fn],
                                        op0=mybir.AluOpType.mult,
                                        op1=mybir.AluOpType.add)
                                    nc.scalar.activation(
                                        e[:, 0:fn], tmp[:, 0:fn],
                                        mybir.ActivationFunctionType.Exp)
                                else:
                                    nc.scalar.activation(
                                        e[:, 0:fn], ss[:, 0:fn],
                                        mybir.ActivationFunctionType.Exp,
                                        scale=scale)
                                es.append(e)
                            for h in range(REP):
                                nc.tensor.matmul(
                                    od[h][:, f0:f0 + fn], v_sb[:, t * P:(t + 1) * P],
                                    es[h][:, 0:fn], start=(t == 0),
                                    stop=(t == nt_c - 1))
                            for h in range(REP):
                                nc.tensor.matmul(
                                    dd[:, f0:f0 + fn],
                                    oneh_sb[:, h * REP:(h + 1) * REP],
                                    es[h][:, 0:fn], start=(t == 0 and h == 0),
                                    stop=(t == nt_c - 1 and h == REP - 1))
                        for h in range(REP):
                            # free the od bank promptly (DVE: ACT is exp-bound)
                            nc.vector.tensor_copy(
                                ohT_sb[:, h * s_len + c * CH: h * s_len + (c + 1) * CH],
                                od[h])
                        r4_sb = npool.tile([REP, CH], F32R, tag="r")
                        with nc.allow_low_precision(
                                reason="f32r is 4-byte storage; rounding only"):
                            nc.vector.reciprocal(r4_sb, dd)
                        for h in range(REP):
                            oh_c = ohT_sb[:, h * s_len + c * CH: h * s_len + (c + 1) * CH]
                            bb = psb.tile([P, CH], F32, tag="bb")
                            nc.tensor.matmul(bb, oneh4_sb[:, h * P:(h + 1) * P],
                                             r4_sb, start=True, stop=True)
                            nc.vector.tensor_mul(oh_c, oh_c, bb)

                    # ------------ Phase O: output projection (reuses sc PSUM)
                    with tc.tile_pool(name="fo", bufs=3) as fopool:
                        for st in range(s_len // P):
                            fo = fopool.tile([P, D], F32, tag="fo")
                            for dc in range(D // CH):
                                pf = pss.tile([P, CH], F32, tag="sc", name=f"pf{st}_{dc}")
                                for j in range(REP):
                                    nc.tensor.matmul(
                                        pf,
                                        ohT_sb[:, j * s_len + st * P: j * s_len + (st + 1) * P],
                                        woT_sb[:, j * D + dc * CH: j * D + (dc + 1) * CH],
                                        start=(j == 0), stop=(j == REP - 1))
                                nc.any.tensor_copy(fo[:, dc * CH:(dc + 1) * CH], pf)
                                nc.sync.dma_start(
                                    out=out[st * P:(st + 1) * P, dc * CH:(dc + 1) * CH],
                                    in_=fo[:, dc * CH:(dc + 1) * CH])

    nc.compile()
    return nc


_PERM = np.concatenate([np.arange(0, DK, 2), np.arange(1, DK, 2)])  # evens, odds


def _prep_device_inputs(x, freqs_cos, freqs_sin, wq_w, wq_b, wk_w, wk_b,
                        wv_w, wv_b, wo_w, s_len=S):
    """Host-side sharding + layout. Returns list of in_maps (len 8)."""
    f32 = np.float32

    def wtile(rows, permute):
        r = rows[_PERM] if permute else rows
        blk = np.ascontiguousarray(r.T).reshape(ND, P, P)      # [dt, p, c]
        return blk.transpose(1, 0, 2).reshape(P, D)            # [p, dt*128+c]

    cs = np.ascontiguousarray(
        np.concatenate([freqs_cos[:s_len].T, freqs_sin[:s_len].T], axis=0), dtype=f32)
    scm = np.ascontiguousarray(
        np.concatenate([freqs_sin[:s_len].T, freqs_cos[:s_len].T], axis=0), dtype=f32)
    oneh = np.zeros((P, 4 * REP), dtype=f32)
    oneh4 = np.zeros((REP, REP * P), dtype=f32)
    for h in range(REP):
        oneh[:, h * REP + h] = 1.0
        oneh4[h, h * P:(h + 1) * P] = 1.0
    pp, xx = np.meshgrid(np.arange(P), np.arange(896), indexing="ij")
    mb = np.where(pp <= xx - 384, 0.0, NEG).astype(f32)
    idm = np.eye(P, dtype=f32)

    in_maps = []
    for d in range(NDEV):
        b, g = d // HK, d % HK
        xt = np.ascontiguousarray(x[b, :s_len].T, dtype=f32)
        wblk = np.empty((P, 6 * D), dtype=f32)
        for m in range(REP):
            h = g * REP + m
            wblk[:, m * D:(m + 1) * D] = wtile(wq_w[h * P:(h + 1) * P], True)
        wblk[:, 4 * D:5 * D] = wtile(wk_w[g * P:(g + 1) * P], True)
        wblk[:, 5 * D:6 * D] = wtile(wv_w[g * P:(g + 1) * P], False)
        wot = np.concatenate(
            [np.ascontiguousarray(wo_w[:, (g * REP + j) * P:(g * REP + j + 1) * P].T)
             for j in range(REP)], axis=0).astype(f32)
        bias = np.zeros((P, 6), dtype=f32)
        for m in range(REP):
            h = g * REP + m
            bias[:, m] = wq_b[h * P:(h + 1) * P][_PERM]
        bias[:, 4] = wk_b[g * P:(g + 1) * P][_PERM]
        bias[:, 5] = wv_b[g * P:(g + 1) * P]
        in_maps.append({
            "xT": xt, "W": np.ascontiguousarray(wblk), "woT": wot,
            "CS": cs, "SC": scm, "MB": mb, "ID": idm,
            "BIAS": np.ascontiguousarray(bias),
            "BIAS2": np.ascontiguousarray(np.roll(bias, 64, axis=0)),
            "ONEH": oneh, "ONEH4": oneh4,
        })
    return in_maps


_CACHE = {}


def _get_nc(s_len=S):
    if s_len not in _CACHE:
        _CACHE[s_len] = _build(s_len)
    return _CACHE[s_len]


def kernel(x, freqs_cos, freqs_sin, wq_w, wq_b, wk_w, wk_b, wv_w, wv_b,
           wo_w, wo_b, _trace=False):
    x = np.asarray(x, dtype=np.float32)
    args = [np.asarray(a, dtype=np.float32) for a in
            (freqs_cos, freqs_sin, wq_w, wq_b, wk_w, wk_b, wv_w, wv_b, wo_w)]
    wo_b = np.asarray(wo_b, dtype=np.float32)
    nc = _get_nc(S)
    in_maps = _prep_device_inputs(x, *args)
    res = run_bass_kernel_spmd(nc, in_maps, core_ids=list(range(NDEV)),
                               trace=_trace)
    outf = np.zeros((B, S, D), dtype=np.float32)
    for d in range(NDEV):
        outf[d // HK] += res.results[d]["out"]
    outf += wo_b[None, None, :]
    kernel.last_result = res
    return outf


# revision 24
# speedup vs baseline: 1.9087x; 1.9087x over previous
"""Trainium2 Bass kernel for GQA attention (B=2, S=2048, D=2048, H=16, HK=4).

Sharding: 8 devices = batch(2) x kv-groups(4). Each device owns one batch
element and one GQA group (4 q-heads + 1 kv-head): wq/wk/wv column-parallel,
wo row-parallel (host sums the 4 partials per batch element).

Device kernel (all matmuls fp32r):
  - QKV projection in [dk, s] orientation (weights stationary, xT moving)
  - RoPE on DVE with an even/odd dk permutation folded into the weights
    host-side (partitions 0:64 = real, 64:128 = imag)
  - scores computed transposed [sk, sq] so softmax needs no transposes;
    exp on ACT reads PSUM directly; no max-subtraction (scores are O(1)
    for this problem's scale)
  - denominators: one-hot-column lhsT matmuls accumulate all 4 heads'
    row-sums into rows 0-3 of a single PSUM bank; one 4-row reciprocal;
    per-head broadcast via a K=4 one-hot matmul
  - causal: upper-triangle score tiles skipped entirely; diagonal tiles
    get an additive -1e9 mask before exp
  - wo row-parallel matmul on device; host adds wo_b and reduces groups
"""

import math

import numpy as np

import concourse.bacc as bacc
import concourse.tile as tile
from concourse import mybir
from concourse.bass_utils import run_bass_kernel_spmd

B, S, D = 2, 2048, 2048
H, HK, DK = 16, 4, 128
REP = H // HK  # 4 q-heads per kv head
NDEV = 8
P = 128
CH = 512            # s-chunk (matmul moving size)
ND = D // P         # 16 d-tiles
F32 = mybir.dt.float32
F32R = mybir.dt.float32r
NEG = -1.0e9


def _build(s_len=S, reps=1):
    """Build the per-device Bass program (SPMD: same program on all cores).

    reps>1 repeats the whole computation (timing only)."""
    nch = s_len // CH          # s-chunks
    scale = 1.0 / math.sqrt(DK)

    nc = bacc.Bacc("TRN2", target_bir_lowering=False, debug=False,
                   enable_asserts=False, num_devices=1)
    xT = nc.dram_tensor("xT", [D, s_len], F32R, kind="ExternalInput").ap()
    W = nc.dram_tensor("W", [P, 6 * D], F32R, kind="ExternalInput").ap()
    woT = nc.dram_tensor("woT", [REP * DK, D], F32R, kind="ExternalInput").ap()
    CSt = nc.dram_tensor("CS", [P, s_len], F32, kind="ExternalInput").ap()
    SCt = nc.dram_tensor("SC", [P, s_len], F32, kind="ExternalInput").ap()
    MBt = nc.dram_tensor("MB", [P, 896], F32, kind="ExternalInput").ap()
    IDt = nc.dram_tensor("ID", [P, P], F32, kind="ExternalInput").ap()
    BIAS = nc.dram_tensor("BIAS", [P, 6], F32, kind="ExternalInput").ap()
    BIAS2 = nc.dram_tensor("BIAS2", [P, 6], F32, kind="ExternalInput").ap()
    ONEH = nc.dram_tensor("ONEH", [P, 4 * REP], F32R, kind="ExternalInput").ap()
    ONEH4 = nc.dram_tensor("ONEH4", [REP, REP * P], F32R, kind="ExternalInput").ap()
    out = nc.dram_tensor("out", [s_len, D], F32, kind="ExternalOutput").ap()

    with tile.TileContext(nc) as tc:
      for _rep in range(reps):
        with tc.tile_pool(name="consts", bufs=1) as consts, \
             tc.tile_pool(name="qkv", bufs=1) as qkpool:
            cs_sb = consts.tile([P, s_len], F32)
            sc_sb = consts.tile([P, s_len], F32)
            mb_sb = consts.tile([P, 896], F32)
            id_sb = consts.tile([P, P], F32)
            bias_sb = consts.tile([P, 6], F32)
            bias2_sb = consts.tile([P, 6], F32)
            oneh_sb = consts.tile([P, 4 * REP], F32R)
            oneh4_sb = consts.tile([REP, REP * P], F32R)

            qk_sb = qkpool.tile([P, 5 * s_len], F32R)  # roped q heads 0..3, k at block 4
            v_sb = qkpool.tile([P, s_len], F32R)       # [s-in-tile, dk] tiles along free

            # ---------------- Phase P: QKV projection + RoPE + v transpose
            # SP DMA order: W[m=0] interleaved with x[q=0], tabs, W[1..5],
            # then x[q] + tabs per chunk. W fully resident (6 MB); x
            # double-buffered in quarter tiles (chunk-sized).
            with tc.tile_pool(name="xh", bufs=2) as xpool, \
                 tc.tile_pool(name="wst", bufs=1) as wpool, \
                 tc.tile_pool(name="rope", bufs=3) as rpool, \
                 tc.tile_pool(name="vT", bufs=1) as vtpool, \
                 tc.tile_pool(name="pp", bufs=4, space="PSUM") as pps, \
                 tc.tile_pool(name="pt", bufs=2, space="PSUM") as pts:
                vT_sb = vtpool.tile([P, s_len], F32)
                w_sb = wpool.tile([P, 6 * D], F32R)

                def load_xq(q, dts=None):
                    xq = xpool.tile([P, ND * CH], F32R, tag="x", name=f"xq{q}")
                    for dt in (range(ND) if dts is None else dts):
                        nc.sync.dma_start(
                            out=xq[:, dt * CH:(dt + 1) * CH],
                            in_=xT[dt * P:(dt + 1) * P, q * CH:(q + 1) * CH])
                    return xq

                def load_x_slices(xq, q, dts):
                    for dt in dts:
                        nc.sync.dma_start(
                            out=xq[:, dt * CH:(dt + 1) * CH],
                            in_=xT[dt * P:(dt + 1) * P, q * CH:(q + 1) * CH])

                def load_tabs(c):
                    nc.sync.dma_start(out=cs_sb[:, c * CH:(c + 1) * CH],
                                      in_=CSt[:, c * CH:(c + 1) * CH])
                    nc.sync.dma_start(out=sc_sb[:, c * CH:(c + 1) * CH],
                                      in_=SCt[:, c * CH:(c + 1) * CH])

                # interleave W[m=0] sub-loads with the first x slices so the
                # first k-step can issue ~2us in (each W chunk covers 4 dt)
                nc.sync.dma_start(out=w_sb[:, 0:CH], in_=W[:, 0:CH])
                xq = load_xq(0, dts=[0])
                for wc in range(1, 4):
                    load_x_slices(xq, 0, range((wc - 1) * 4 + 1, wc * 4 + 1))
                    nc.sync.dma_start(out=w_sb[:, wc * CH:(wc + 1) * CH],
                                      in_=W[:, wc * CH:(wc + 1) * CH])
                load_x_slices(xq, 0, range(13, ND))
                nc.sync.dma_start(out=bias_sb, in_=BIAS)
                nc.sync.dma_start(out=bias2_sb, in_=BIAS2)
                load_tabs(0)
                for m in range(1, 6):
                    nc.sync.dma_start(out=w_sb[:, m * D:(m + 1) * D],
                                      in_=W[:, m * D:(m + 1) * D])
                nc.sync.dma_start(out=id_sb, in_=IDt)

                for c in range(nch):
                    if c > 0:
                        xq = load_xq(c)
                        load_tabs(c)
                    if c == (1 if nch > 1 else 0):
                        nc.sync.dma_start(out=mb_sb, in_=MBt)
                        nc.sync.dma_start(out=oneh_sb, in_=ONEH)
                        nc.sync.dma_start(out=oneh4_sb, in_=ONEH4)
                    for m in range(6):
                        ps = pps.tile([P, CH], F32, tag="pp")
                        for dt in range(ND):
                            nc.tensor.matmul(
                                ps, w_sb[:, m * D + dt * P: m * D + (dt + 1) * P],
                                xq[:, dt * CH:(dt + 1) * CH],
                                start=(dt == 0), stop=(dt == ND - 1))
                        if m < 5:
                            # RoPE: partitions 0:64 real (qr), 64:128 imag (qi).
                            # U[0:64]=(qr+b0)cos  U[64:]=(qr+b0)sin
                            # V[0:64]=(qi+b1)sin  V[64:]=(qi+b1)cos
                            # All SBUF operands of each op share a base
                            # partition (walrus NCC_IBIR297); the PSUM
                            # input's base is independent.
                            cs_c = cs_sb[:, c * CH:(c + 1) * CH]
                            sc_c = sc_sb[:, c * CH:(c + 1) * CH]
                            add, mult = mybir.AluOpType.add, mybir.AluOpType.mult
                            u = rpool.tile([P, CH], F32, tag="p1")
                            v = rpool.tile([P, CH], F32, tag="p2")
                            nc.vector.scalar_tensor_tensor(
                                u[0:64], ps[0:64], bias_sb[0:64, m:m + 1],
                                cs_c[0:64], op0=add, op1=mult)
                            nc.vector.scalar_tensor_tensor(
                                u[64:128], ps[0:64], bias2_sb[64:128, m:m + 1],
                                cs_c[64:128], op0=add, op1=mult)
                            nc.vector.scalar_tensor_tensor(
                                v[0:64], ps[64:128], bias2_sb[0:64, m:m + 1],
                                sc_c[0:64], op0=add, op1=mult)
                            nc.vector.scalar_tensor_tensor(
                                v[64:128], ps[64:128], bias_sb[64:128, m:m + 1],
                                sc_c[64:128], op0=add, op1=mult)
                            dst = qk_sb[:, m * s_len + c * CH: m * s_len + (c + 1) * CH]
                            nc.vector.tensor_sub(dst[0:64], u[0:64], v[0:64])
                            nc.vector.tensor_add(dst[64:128], u[64:128], v[64:128])
                        else:
                            nc.scalar.add(out=vT_sb[:, c * CH:(c + 1) * CH],
                                          in_=ps, add=bias_sb[:, m:m + 1])
                    for tt in range(c * (CH // P), (c + 1) * (CH // P)):
                        ptr = pts.tile([P, P], F32, tag="pt")
                        nc.tensor.transpose(ptr, vT_sb[:, tt * P:(tt + 1) * P], id_sb)
                        nc.any.tensor_copy(v_sb[:, tt * P:(tt + 1) * P], ptr)

            # ---------------- Phase A: attention
            with tc.tile_pool(name="oh", bufs=1) as ohpool, \
                 tc.tile_pool(name="wo", bufs=1) as wopool:
                ohT_sb = ohpool.tile([P, REP * s_len], F32R)
                woT_sb = wopool.tile([P, REP * D], F32R)
                for j in range(REP):
                    nc.sync.dma_start(out=woT_sb[:, j * D:(j + 1) * D],
                                      in_=woT[j * P:(j + 1) * P, :])

                with tc.tile_pool(name="ew", bufs=8) as epool, \
                     tc.tile_pool(name="mt", bufs=4) as tpool, \
                     tc.tile_pool(name="nrm", bufs=4) as npool, \
                     tc.tile_pool(name="ps_s", bufs=2, space="PSUM") as pss, \
                     tc.tile_pool(name="ps_o", bufs=4, space="PSUM") as pso, \
                     tc.tile_pool(name="ps_b", bufs=1, space="PSUM") as psb, \
                     tc.tile_pool(name="ps_d", bufs=1, space="PSUM") as psd:
                    for c in range(nch):
                        nt_c = (c + 1) * (CH // P)  # causal sk tiles for this chunk
                        od = [pso.tile([P, CH], F32, tag="od", name=f"od{c}_{h}")
                              for h in range(REP)]
                        dd = psd.tile([REP, CH], F32, tag="dd")
                        for t in range(nt_c):
                            # causal: slice the moving (sq) range down to the
                            # valid region, >=256 wide (fp32r rate needs
                            # ap_size >= 256). delta = c*CH - t*P.
                            delta = c * CH - t * P
                            if delta >= 256:
                                f0, fn = 0, CH
                            elif delta >= 0:
                                f0, fn = 0, CH
                            elif delta == -128:
                                f0, fn = 128, 384
                            else:  # -256, -384
                                f0, fn = 256, 256
                            diag = (t >= nt_c - 4)
                            es = []
                            for h in range(REP):
                                ss = pss.tile([P, CH], F32, tag="sc")
                                nc.tensor.matmul(
                                    ss[:, 0:fn],
                                    qk_sb[:, 4 * s_len + t * P: 4 * s_len + (t + 1) * P],
                                    qk_sb[:, h * s_len + c * CH + f0: h * s_len + c * CH + f0 + fn],
                                    start=True, stop=True)
                                e = epool.tile([P, CH], F32R, tag="e")
                                if diag:
                                    off = delta + 384 + f0
                                    tmp = tpool.tile([P, CH], F32, tag="mt")
                                    nc.vector.scalar_tensor_tensor(
                                        tmp[:, 0:fn], ss[:, 0:fn], scale,
                                        mb_sb[:, off:off + fn],
                                        op0=mybir.AluOpType.mult,
                                        op1=mybir.AluOpType.add)
                                    nc.scalar.activation(
                                        e[:, 0:fn], tmp[:, 0:fn],
                                        mybir.ActivationFunctionType.Exp)
                                else:
                                    nc.scalar.activation(
                                        e[:, 0:fn], ss[:, 0:fn],
                                        mybir.ActivationFunctionType.Exp,
                                        scale=scale)
                                es.append(e)
                            for h in range(REP):
                                nc.tensor.matmul(
                                    od[h][:, f0:f0 + fn], v_sb[:, t * P:(t + 1) * P],
                                    es[h][:, 0:fn], start=(t == 0),
                                    stop=(t == nt_c - 1))
                            for h in range(REP):
                                nc.tensor.matmul(
                                    dd[:, f0:f0 + fn],
                                    oneh_sb[:, h * REP:(h + 1) * REP],
                                    es[h][:, 0:fn], start=(t == 0 and h == 0),
                                    stop=(t == nt_c - 1 and h == REP - 1))
                        for h in range(REP):
                            # free the od bank promptly (DVE: ACT is exp-bound)
                            nc.vector.tensor_copy(
                                ohT_sb[:, h * s_len + c * CH: h * s_len + (c + 1) * CH],
                                od[h])
                        r4_sb = npool.tile([REP, CH], F32R, tag="r")
                        with nc.allow_low_precision(
                                reason="f32r is 4-byte storage; rounding only"):
                            nc.vector.reciprocal(r4_sb, dd)
                        for h in range(REP):
                            oh_c = ohT_sb[:, h * s_len + c * CH: h * s_len + (c + 1) * CH]
                            bb = psb.tile([P, CH], F32, tag="bb")
                            nc.tensor.matmul(bb, oneh4_sb[:, h * P:(h + 1) * P],
                                             r4_sb, start=True, stop=True)
                            nc.vector.tensor_mul(oh_c, oh_c, bb)

                    # ------------ Phase O: output projection (reuses sc PSUM)
                    with tc.tile_pool(name="fo", bufs=3) as fopool:
                        for st in range(s_len // P):
                            fo = fopool.tile([P, D], F32, tag="fo")
                            for dc in range(D // CH):
                                pf = pss.tile([P, CH], F32, tag="sc", name=f"pf{st}_{dc}")
                                for j in range(REP):
                                    nc.tensor.matmul(
                                        pf,
                                        ohT_sb[:, j * s_len + st * P: j * s_len + (st + 1) * P],
                                        woT_sb[:, j * D + dc * CH: j * D + (dc + 1) * CH],
                                        start=(j == 0), stop=(j == REP - 1))
                                nc.any.tensor_copy(fo[:, dc * CH:(dc + 1) * CH], pf)
                                nc.sync.dma_start(
                                    out=out[st * P:(st + 1) * P, dc * CH:(dc + 1) * CH],
                                    in_=fo[:, dc * CH:(dc + 1) * CH])

    nc.compile()
    return nc


_PERM = np.concatenate([np.arange(0, DK, 2), np.arange(1, DK, 2)])  # evens, odds


def _prep_device_inputs(x, freqs_cos, freqs_sin, wq_w, wq_b, wk_w, wk_b,
                        wv_w, wv_b, wo_w, s_len=S):
    """Host-side sharding + layout. Returns list of in_maps (len 8)."""
    f32 = np.float32

    def wtile(rows, permute):
        r = rows[_PERM] if permute else rows
        blk = np.ascontiguousarray(r.T).reshape(ND, P, P)      # [dt, p, c]
        return blk.transpose(1, 0, 2).reshape(P, D)            # [p, dt*128+c]

    cs = np.ascontiguousarray(
        np.concatenate([freqs_cos[:s_len].T, freqs_sin[:s_len].T], axis=0), dtype=f32)
    scm = np.ascontiguousarray(
        np.concatenate([freqs_sin[:s_len].T, freqs_cos[:s_len].T], axis=0), dtype=f32)
    oneh = np.zeros((P, 4 * REP), dtype=f32)
    oneh4 = np.zeros((REP, REP * P), dtype=f32)
    for h in range(REP):
        oneh[:, h * REP + h] = 1.0
        oneh4[h, h * P:(h + 1) * P] = 1.0
    pp, xx = np.meshgrid(np.arange(P), np.arange(896), indexing="ij")
    mb = np.where(pp <= xx - 384, 0.0, NEG).astype(f32)
    idm = np.eye(P, dtype=f32)

    in_maps = []
    for d in range(NDEV):
        b, g = d // HK, d % HK
        xt = np.ascontiguousarray(x[b, :s_len].T, dtype=f32)
        wblk = np.empty((P, 6 * D), dtype=f32)
        for m in range(REP):
            h = g * REP + m
            wblk[:, m * D:(m + 1) * D] = wtile(wq_w[h * P:(h + 1) * P], True)
        wblk[:, 4 * D:5 * D] = wtile(wk_w[g * P:(g + 1) * P], True)
        wblk[:, 5 * D:6 * D] = wtile(wv_w[g * P:(g + 1) * P], False)
        wot = np.concatenate(
            [np.ascontiguousarray(wo_w[:, (g * REP + j) * P:(g * REP + j + 1) * P].T)
             for j in range(REP)], axis=0).astype(f32)
        bias = np.zeros((P, 6), dtype=f32)
        for m in range(REP):
            h = g * REP + m
            bias[:, m] = wq_b[h * P:(h + 1) * P][_PERM]
        bias[:, 4] = wk_b[g * P:(g + 1) * P][_PERM]
        bias[:, 5] = wv_b[g * P:(g + 1) * P]
        in_maps.append({
            "xT": xt, "W": np.ascontiguousarray(wblk), "woT": wot,
            "CS": cs, "SC": scm, "MB": mb, "ID": idm,
            "BIAS": np.ascontiguousarray(bias),
            "BIAS2": np.ascontiguousarray(np.roll(bias, 64, axis=0)),
            "ONEH": oneh, "ONEH4": oneh4,
        })
    return in_maps


_CACHE = {}


def _get_nc(s_len=S):
    if s_len not in _CACHE:
        _CACHE[s_len] = _build(s_len)
    return _CACHE[s_len]


def kernel(x, freqs_cos, freqs_sin, wq_w, wq_b, wk_w, wk_b, wv_w, wv_b,
           wo_w, wo_b, _trace=False):
    x = np.asarray(x, dtype=np.float32)
    args = [np.asarray(a, dtype=np.float32) for a in
            (freqs_cos, freqs_sin, wq_w, wq_b, wk_w, wk_b, wv_w, wv_b, wo_w)]
    wo_b = np.asarray(wo_b, dtype=np.float32)
    nc = _get_nc(S)
    in_maps = _prep_device_inputs(x, *args)
    res = run_bass_kernel_spmd(nc, in_maps, core_ids=list(range(NDEV)),
                               trace=_trace)
    outf = np.zeros((B, S, D), dtype=np.float32)
    for d in range(NDEV):
        outf[d // HK] += res.results[d]["out"]
    outf += wo_b[None, None, :]
    kernel.last_result = res
    return outf
